# revision 11
# baseline (speedup 1.0000x reference)
"""Multi-head attention (B=2, L=2048, dim=1024, 16 heads) on 8 Trainium2 cores.

Sharding: 8 cores = 2 (batch) x 4 (head groups of 4 heads). Each core runs an
identical Bass program on its own slice (SPMD, no collectives); the host sums
the 4 per-head-group partial projection outputs per batch and adds the bias.

Per-core dataflow (all fp32 in SBUF/PSUM; matmuls issued as float32r, the
full-rate fp32 PE mode):
  xT [1024, 2048]  (x[b] transposed, c-major)
  qT/kT per head pair: [128 (2 heads x 64d), 2048 tok] feature-major
  V token-major: [128 tok, 4 heads, 64]
  ST[k, q] = kT.T @ qT  (K=64 contraction, 2 heads row-packed in the PE array)
  PT = exp(ST / 8)  on ScalarE, PSUM -> SBUF
  OT[d, q] += V.T @ PT  (col-packed head pairs), norm[q] += ones.T @ PT
  OT_norm = OT * bcast(1/norm)  on DVE
  out[tok, c] = OT_norm.T @ wpT  (contract 256 head channels)
"""

import os
import numpy as np

B, L, C = 2, 2048, 1024
H, D = 16, 64
HL = 4            # heads per core (local)
PAIRS = 2         # head pairs per core
CT = C // 128     # 8 contraction tiles for the projections
TOK = L // 128    # 16 key-token tiles
QW = 512          # query tile width
QS = L // QW      # 4 query tiles
NCORES = 8

_cache = {}


def _build_nc():
    import concourse.bass as bass
    import concourse.mybir as mybir
    import concourse.tile as tile
    from concourse import bacc

    F32 = mybir.dt.float32
    F32R = mybir.dt.float32r
    EXP = mybir.ActivationFunctionType.Exp

    nc = bacc.Bacc("TRN2", target_bir_lowering=False, debug=False,
                   num_devices=NCORES)

    xT = nc.declare_dram_parameter("xT", [C, L], F32R, isOutput=False)
    wT = nc.declare_dram_parameter("wT", [C, 3 * HL * D], F32R, isOutput=False)
    wpT = nc.declare_dram_parameter("wpT", [HL * D, C], F32R, isOutput=False)
    out = nc.declare_dram_parameter("out", [L, C], F32, isOutput=True)

    with tile.TileContext(nc) as tc:
        from contextlib import ExitStack
        with ExitStack() as ctx:
            # pools that live for the whole kernel
            qkpool = ctx.enter_context(tc.tile_pool(name="qk", bufs=1))
            vpool = ctx.enter_context(tc.tile_pool(name="v", bufs=1))
            wppool = ctx.enter_context(tc.tile_pool(name="wp", bufs=1))
            psA = ctx.enter_context(tc.tile_pool(name="psA", bufs=2, space="PSUM"))
            psS = ctx.enter_context(tc.tile_pool(name="psS", bufs=2, space="PSUM"))
            psO = ctx.enter_context(tc.tile_pool(name="psO", bufs=2, space="PSUM"))

            wp_t = []
            for h in range(HL):
                t = wppool.tile([64, C], F32R, name=f"wp{h}", tag=f"wp{h}")
                nc.sync.dma_start(out=t, in_=wpT[D * h:D * (h + 1), :])
                wp_t.append(t)

            # ---- phase 1: QKV projections (x/w pools scoped to this phase) ---
            with tc.tile_pool(name="w", bufs=1) as wpool, \
                 tc.tile_pool(name="x", bufs=1) as xpool:
                x_t = []
                for i in range(CT):
                    t = xpool.tile([128, L], F32R, tag=f"x{i}")
                    nc.sync.dma_start(out=t, in_=xT[128 * i:128 * (i + 1), :])
                    x_t.append(t)
                w_t = []
                for i in range(CT):
                    t = wpool.tile([128, 3 * HL * D], F32R, tag=f"w{i}")
                    nc.sync.dma_start(out=t, in_=wT[128 * i:128 * (i + 1), :])
                    w_t.append(t)

                # V token-major: v[t] = [128 tok, HL, D+1] (ones col fused)
                ones_s = vpool.tile([128, HL, 1], F32, name="ones_s", tag="ones_s")
                nc.vector.memset(ones_s, 1.0)
                v_t = []
                for t in range(TOK):
                    ps = psA.tile([128, HL * D], F32, tag="ps")
                    for c in range(CT):
                        nc.tensor.matmul(
                            ps,
                            lhsT=x_t[c][:, 128 * t:128 * (t + 1)],
                            rhs=w_t[c][:, 2 * HL * D:3 * HL * D],
                            start=(c == 0), stop=(c == CT - 1),
                        )
                    vt = vpool.tile([128, HL, D + 1], F32R, name=f"v{t}", tag=f"v{t}")
                    nc.vector.tensor_copy(out=vt[:, :, D:D + 1], in_=ones_s)
                    nc.vector.tensor_copy(
                        out=vt[:, :, 0:D],
                        in_=ps.rearrange("p (h d) -> p h d", h=HL),
                    )
                    v_t.append(vt)

                # Q/K feature-major per pair: [128 (2h x 64d), L]
                qk_t = {}
                for p in range(PAIRS):
                    for j, nm in ((0, "q"), (1, "k")):
                        dst = qkpool.tile([128, L], F32R, tag=f"{nm}{p}")
                        qk_t[(nm, p)] = dst
                        for ns in range(QS):
                            ps = psA.tile([128, QW], F32, tag="ps")
                            for c in range(CT):
                                nc.tensor.matmul(
                                    ps,
                                    lhsT=w_t[c][:, j * HL * D + 128 * p:
                                                j * HL * D + 128 * (p + 1)],
                                    rhs=x_t[c][:, QW * ns:QW * (ns + 1)],
                                    start=(c == 0), stop=(c == CT - 1),
                                )
                            nc.vector.tensor_copy(
                                out=dst[:, QW * ns:QW * (ns + 1)], in_=ps)

            # ---- phase 2 pools (reuse the x/w SBUF space) --------------------
            otpool = ctx.enter_context(tc.tile_pool(name="ot", bufs=1))
            ptpool = ctx.enter_context(tc.tile_pool(name="pt", bufs=4))
            rpool = ctx.enter_context(tc.tile_pool(name="r", bufs=2))
            obpool = ctx.enter_context(tc.tile_pool(name="ob", bufs=3))

            # ---- attention ---------------------------------------------------
            # One normalized [64, L] output tile per local head; O matmuls are
            # M=65 (64 V columns + a ones column -> softmax denominator in
            # psum row 64); exactly one accumulation group per PSUM bank.
            ot_sb = [otpool.tile([64, L], F32R, name=f"oth{h}", tag=f"oth{h}")
                     for h in range(HL)]
            for p in range(PAIRS):
                kT = qk_t[("k", p)]
                qT = qk_t[("q", p)]
                for qs in range(QS):
                    ot_a = psO.tile([65, QW], F32, name="ot_a", tag="ot")
                    ot_b = psO.tile([65, QW], F32, name="ot_b", tag="ot")
                    for kb in range(TOK):
                        st = psS.tile([128, 2 * QW], F32, tag="st")
                        # scores for both heads of the pair (row-packed K=64)
                        nc.tensor.matmul(
                            st[:, 0:QW],
                            lhsT=kT[0:64, 128 * kb:128 * (kb + 1)],
                            rhs=qT[0:64, QW * qs:QW * (qs + 1)],
                            start=True, stop=True,
                        )
                        nc.tensor.matmul(
                            st[:, QW:2 * QW],
                            lhsT=kT[64:128, 128 * kb:128 * (kb + 1)],
                            rhs=qT[64:128, QW * qs:QW * (qs + 1)],
                            start=True, stop=True,
                        )
                        pt = ptpool.tile([128, 2 * QW], F32R, tag="pt")
                        nc.scalar.activation(out=pt, in_=st, func=EXP, scale=0.125)
                        # O accumulation (64 V cols + ones col per head)
                        nc.tensor.matmul(
                            ot_a,
                            lhsT=v_t[kb][:, 2 * p, :],
                            rhs=pt[:, 0:QW],
                            start=(kb == 0), stop=(kb == TOK - 1),
                        )
                        nc.tensor.matmul(
                            ot_b,
                            lhsT=v_t[kb][:, 2 * p + 1, :],
                            rhs=pt[:, QW:2 * QW],
                            start=(kb == 0), stop=(kb == TOK - 1),
                        )
                    # normalize: ot_sb[h][:, qs] = ot[0:64] * bcast(1/ot[64])
                    rsb = rpool.tile([65, 2 * QW], F32, tag="rsb")
                    nc.vector.reciprocal(out=rsb[64:65, 0:QW], in_=ot_a[64:65, :])
                    nc.vector.reciprocal(out=rsb[64:65, QW:2 * QW], in_=ot_b[64:65, :])
                    rbc = rpool.tile([64, 2 * QW], F32, tag="rbc")
                    pstep = rsb.ap[0][0]
                    nc.sync.dma_start(out=rbc[:, 0:QW], in_=bass.AP(
                        tensor=rsb.tensor, offset=rsb.offset + 64 * pstep,
                        ap=[[pstep, 1], [0, 64], [1, QW]]))
                    nc.sync.dma_start(out=rbc[:, QW:2 * QW], in_=bass.AP(
                        tensor=rsb.tensor, offset=rsb.offset + 64 * pstep + QW,
                        ap=[[pstep, 1], [0, 64], [1, QW]]))
                    nc.vector.tensor_mul(
                        out=ot_sb[2 * p][:, QW * qs:QW * (qs + 1)],
                        in0=ot_a[0:64, :], in1=rbc[:, 0:QW])
                    nc.vector.tensor_mul(
                        out=ot_sb[2 * p + 1][:, QW * qs:QW * (qs + 1)],
                        in0=ot_b[0:64, :], in1=rbc[:, QW:2 * QW])

            # ---- output projection (contract 4 heads x 64 channels) ----------
            for t in range(TOK):
                ob = obpool.tile([128, C], F32, tag="ob")
                for nh in range(C // QW):
                    ps = psA.tile([128, QW], F32, tag="ps")
                    for h in range(HL):
                        nc.tensor.matmul(
                            ps,
                            lhsT=ot_sb[h][:, 128 * t:128 * (t + 1)],
                            rhs=wp_t[h][:, QW * nh:QW * (nh + 1)],
                            start=(h == 0), stop=(h == HL - 1),
                        )
                    nc.vector.tensor_copy(out=ob[:, QW * nh:QW * (nh + 1)], in_=ps)
                nc.sync.dma_start(out=out[128 * t:128 * (t + 1), :], in_=ob)

    nc.compile()
    return nc


def _get_nc():
    if "nc" not in _cache:
        _cache["nc"] = _build_nc()
    return _cache["nc"]


def kernel(x, w_qkv, w_proj, b_proj):
    from concourse.bass_utils import run_bass_kernel_spmd

    x = np.asarray(x, dtype=np.float32)
    w_qkv = np.asarray(w_qkv, dtype=np.float32)
    w_proj = np.asarray(w_proj, dtype=np.float32)
    b_proj = np.asarray(b_proj, dtype=np.float32)

    nc = _get_nc()
    in_maps = []
    for core in range(NCORES):
        b, g = divmod(core, 4)
        rows = np.concatenate([
            np.arange(C * j + HL * D * g, C * j + HL * D * (g + 1))
            for j in range(3)
        ])
        in_maps.append({
            "xT": np.ascontiguousarray(x[b].T),
            "wT": np.ascontiguousarray(w_qkv[rows].T),
            "wpT": np.ascontiguousarray(w_proj[:, HL * D * g:HL * D * (g + 1)].T),
        })

    res = run_bass_kernel_spmd(
        nc, in_maps, list(range(NCORES)),
        trace=bool(os.environ.get("KERNEL_TRACE")),
    )
    _cache["last_results"] = res

    out = np.empty((B, L, C), dtype=np.float32)
    for b in range(B):
        acc = res.results[4 * b]["out"].astype(np.float32)
        for g in range(1, 4):
            acc = acc + res.results[4 * b + g]["out"]
        out[b] = acc + b_proj[None, :]
    return out
